# revision 1
# baseline (speedup 1.0000x reference)
"""Chunked sliding-window attention (B=2, T=8192, H=16, Dh=128, W=256) on 8
Trainium2 NeuronCores.

Sharding: 8 cores = 2 (batch) x 4 (head groups of 4 heads). Each core computes
q/k/v projections for its 512-wide slice of the 2048 projection dims, RoPE,
chunked attention for its 4 heads, and a partial output projection over its
512 rows of Wo^T. The host sums the 4 partial outputs per batch element.

Device layouts (host-prepared):
  xt   [128, 16, T]   x^T tiles: xt[p, kt, t] = x[b, t, kt*128+p]        (bf16)
  wq/wk[128, 16, 512] (Wq_perm)^T slice, rope-split row permutation      (bf16)
  wv   [128, 16, 512] Wv^T slice (unpermuted)                            (bf16)
  wo   [128, 4, 2048] Wo^T rows for this core's 512 dims                 (bf16)
  ccat [128, T]       [cos; cos] rope table (freq idx on partitions)     (bf16)
  scat [128, T]       [-sin; sin]                                        (bf16)
  mask [128, 2, 256]  transposed causal 0/1 masks for own-chunk kv tiles (bf16)

The rope row-permutation maps interleaved (re,im) pairs to split layout
(re block rows 0..63, im rows 64..127 per head); applied identically to q and
k it leaves scores invariant, and makes rope unit-stride on chip.

Attention is computed in transposed-score layout [kv, q]: softmax denominator
comes from an all-ones stationary matmul (broadcasts the per-q denominator
across all 128 partitions), masking is a 0/1 multiply after exp.
"""

import os

import numpy as np
import ml_dtypes

N_HEAD = 16
HEAD_DIM = 128
WINDOW = 256
THETA = 10000.0
B = 2
T = 8192
DM = 2048
KT = DM // 128      # 16 contraction tiles
HS = 4              # heads per core
DS = HS * HEAD_DIM  # 512 projection dims per core
BLK = 512           # tokens per pipeline block (2 chunks)
CH = WINDOW         # 256
SCALE = float(HEAD_DIM) ** -0.5

LAST_EXEC_NS = None
_NC = None

bf16 = ml_dtypes.bfloat16


def _build_nc(t_len=T):
    from contextlib import ExitStack

    import concourse.tile as tile
    from concourse import bacc, mybir

    fp32 = mybir.dt.float32
    b16 = mybir.dt.bfloat16

    nb = t_len // BLK
    nc = bacc.Bacc("TRN2", target_bir_lowering=False, debug=False)

    xt = nc.dram_tensor("xt", [128, KT, t_len], b16, kind="ExternalInput").ap()
    wq = nc.dram_tensor("wq", [128, KT, DS], b16, kind="ExternalInput").ap()
    wk = nc.dram_tensor("wk", [128, KT, DS], b16, kind="ExternalInput").ap()
    wv = nc.dram_tensor("wv", [128, KT, DS], b16, kind="ExternalInput").ap()
    wo = nc.dram_tensor("wo", [128, HS, DM], b16, kind="ExternalInput").ap()
    ccat = nc.dram_tensor("ccat", [128, t_len], b16, kind="ExternalInput").ap()
    scat = nc.dram_tensor("scat", [128, t_len], b16, kind="ExternalInput").ap()
    mask = nc.dram_tensor("mask", [128, 2, CH], b16, kind="ExternalInput").ap()
    y = nc.dram_tensor("y", [t_len, DM], fp32, kind="ExternalOutput").ap()

    Exp = mybir.ActivationFunctionType.Exp

    with tile.TileContext(nc) as tc, ExitStack() as ctx:
        const = ctx.enter_context(tc.tile_pool(name="const", bufs=1))
        xt_p = ctx.enter_context(tc.tile_pool(name="xtp", bufs=2))
        raw_p = ctx.enter_context(tc.tile_pool(name="rawp", bufs=3))
        swp_p = ctx.enter_context(tc.tile_pool(name="swpp", bufs=3))
        tmp_p = ctx.enter_context(tc.tile_pool(name="tmpp", bufs=3))
        qr_p = ctx.enter_context(tc.tile_pool(name="qrp", bufs=8))
        kr_p = ctx.enter_context(tc.tile_pool(name="krp", bufs=10))
        v_p = ctx.enter_context(tc.tile_pool(name="vp", bufs=10))
        e_p = ctx.enter_context(tc.tile_pool(name="ep", bufs=8))
        rc_p = ctx.enter_context(tc.tile_pool(name="rcp", bufs=4))
        ot_p = ctx.enter_context(tc.tile_pool(name="otp", bufs=16))
        y_p = ctx.enter_context(tc.tile_pool(name="yp", bufs=2))
        ps_big = ctx.enter_context(tc.tile_pool(name="psbig", bufs=3, space="PSUM"))
        ps_st = ctx.enter_context(tc.tile_pool(name="psst", bufs=2, space="PSUM"))
        ps_do = ctx.enter_context(tc.tile_pool(name="psdo", bufs=3, space="PSUM"))

        wq_sb = const.tile([128, KT, DS], b16)
        nc.sync.dma_start(wq_sb, wq)
        wk_sb = const.tile([128, KT, DS], b16)
        nc.sync.dma_start(wk_sb, wk)
        wv_sb = const.tile([128, KT, DS], b16)
        nc.sync.dma_start(wv_sb, wv)
        wo_sb = const.tile([128, HS, DM], b16)
        nc.sync.dma_start(wo_sb, wo)
        ccat_sb = const.tile([128, t_len], b16)
        nc.sync.dma_start(ccat_sb, ccat)
        scat_sb = const.tile([128, t_len], b16)
        nc.sync.dma_start(scat_sb, scat)
        mask_sb = const.tile([128, 2, CH], b16)
        nc.sync.dma_start(mask_sb, mask)
        ones_sb = const.tile([128, 128], b16)
        nc.vector.memset(ones_sb, 1.0)

        prev_k = [None] * HS
        prev_v = [None, None]
        for blk in range(nb):
            t0 = blk * BLK
            xt_sb = xt_p.tile([128, KT, BLK], b16, tag="xt")
            nc.sync.dma_start(xt_sb, xt[:, :, t0 : t0 + BLK])

            c_sl = ccat_sb[:, t0 : t0 + BLK]
            s_sl = scat_sb[:, t0 : t0 + BLK]
            cur_q = []
            cur_k = []
            for h in range(HS):
                for w_sb, dst in ((wq_sb, cur_q), (wk_sb, cur_k)):
                    ps = ps_big.tile([128, BLK], fp32, tag="psbig")
                    for k in range(KT):
                        nc.tensor.matmul(
                            ps,
                            lhsT=w_sb[:, k, h * 128 : (h + 1) * 128],
                            rhs=xt_sb[:, k, :],
                            start=(k == 0),
                            stop=(k == KT - 1),
                        )
                    raw = raw_p.tile([128, BLK], b16, tag="raw")
                    nc.scalar.copy(raw, ps)
                    # swap the (re, im) halves via SBUF->SBUF DMA (DVE lanes
                    # cannot cross partitions)
                    swp = swp_p.tile([128, BLK], b16, tag="swp")
                    nc.sync.dma_start(swp[0:64, :], raw[64:128, :])
                    nc.sync.dma_start(swp[64:128, :], raw[0:64, :])
                    t1 = tmp_p.tile([128, BLK], b16, tag="t1")
                    nc.vector.tensor_mul(t1, raw, c_sl)
                    t2 = tmp_p.tile([128, BLK], b16, tag="t2")
                    nc.vector.tensor_mul(t2, swp, s_sl)
                    if dst is cur_q:
                        rot = qr_p.tile([128, BLK], b16, tag="qr")
                    else:
                        rot = kr_p.tile([128, BLK], b16, tag="kr")
                    nc.vector.tensor_add(rot, t1, t2)
                    dst.append(rot)

            cur_v = []
            for tt in range(4):
                ps = ps_big.tile([128, BLK], fp32, tag="psbig")
                for k in range(KT):
                    nc.tensor.matmul(
                        ps,
                        lhsT=xt_sb[:, k, tt * 128 : (tt + 1) * 128],
                        rhs=wv_sb[:, k, :],
                        start=(k == 0),
                        stop=(k == KT - 1),
                    )
                vt = v_p.tile([128, DS], b16, tag="v")
                nc.vector.tensor_copy(out=vt, in_=ps)
                cur_v.append(vt)

            ot_tiles = {}
            for ci in range(2):
                c = 2 * blk + ci
                qoff = ci * CH
                js = [2, 3] if c == 0 else [0, 1, 2, 3]
                for h in range(HS):
                    q_sl = cur_q[h][:, qoff : qoff + CH]
                    es = []
                    for j in js:
                        if j < 2:
                            if ci == 1:
                                ksrc = cur_k[h][:, j * 128 : (j + 1) * 128]
                            else:
                                ksrc = prev_k[h][:, CH + j * 128 : CH + (j + 1) * 128]
                        else:
                            ksrc = cur_k[h][:, qoff + (j - 2) * 128 : qoff + (j - 1) * 128]
                        st = ps_st.tile([128, CH], fp32, tag="st")
                        nc.tensor.matmul(st, lhsT=ksrc, rhs=q_sl, start=True, stop=True)
                        e = e_p.tile([128, CH], b16, tag="e")
                        nc.scalar.activation(e, st, Exp, scale=SCALE)
                        if j >= 2:
                            nc.vector.tensor_mul(e, e, mask_sb[:, j - 2, :])
                        es.append((j, e))
                    dn = ps_do.tile([128, CH], fp32, tag="do")
                    for i, (j, e) in enumerate(es):
                        nc.tensor.matmul(
                            dn, lhsT=ones_sb, rhs=e,
                            start=(i == 0), stop=(i == len(es) - 1),
                        )
                    ou = ps_do.tile([128, CH], fp32, tag="do")
                    for i, (j, e) in enumerate(es):
                        if j < 2:
                            vsrc = cur_v[j] if ci == 1 else prev_v[j]
                        else:
                            vsrc = cur_v[2 * ci + (j - 2)]
                        nc.tensor.matmul(
                            ou, lhsT=vsrc[:, h * 128 : (h + 1) * 128], rhs=e,
                            start=(i == 0), stop=(i == len(es) - 1),
                        )
                    rc = rc_p.tile([128, CH], fp32, tag="rc")
                    nc.vector.reciprocal(rc, dn)
                    ot = ot_p.tile([128, CH], b16, tag="ot")
                    nc.vector.tensor_mul(ot, ou, rc)
                    ot_tiles[(h, ci)] = ot

            for tt in range(4):
                ci, sub = tt // 2, tt % 2
                ysb = y_p.tile([128, DM], fp32, tag="y")
                for ct in range(4):
                    yps = ps_big.tile([128, 512], fp32, tag="psbig")
                    for h in range(HS):
                        nc.tensor.matmul(
                            yps,
                            lhsT=ot_tiles[(h, ci)][:, sub * 128 : (sub + 1) * 128],
                            rhs=wo_sb[:, h, ct * 512 : (ct + 1) * 512],
                            start=(h == 0),
                            stop=(h == HS - 1),
                        )
                    nc.scalar.copy(ysb[:, ct * 512 : (ct + 1) * 512], yps)
                nc.sync.dma_start(y[t0 + tt * 128 : t0 + (tt + 1) * 128, :], ysb)

            prev_k = cur_k
            prev_v = cur_v[2:4]

    nc.compile()
    return nc


def _rope_perm():
    perm = np.empty(DM, np.int64)
    for h in range(N_HEAD):
        base = h * HEAD_DIM
        perm[base : base + 64] = base + 2 * np.arange(64)
        perm[base + 64 : base + 128] = base + 2 * np.arange(64) + 1
    return perm


def _prep_inputs(x, Wq, Wk, Wv, Wo, t_len=T):
    """Build per-core in_maps. Cores 0-3: batch 0, head groups 0-3; 4-7: batch 1."""
    x = np.asarray(x, dtype=np.float32)
    Wq = np.asarray(Wq, dtype=np.float32)
    Wk = np.asarray(Wk, dtype=np.float32)
    Wv = np.asarray(Wv, dtype=np.float32)
    Wo = np.asarray(Wo, dtype=np.float32)
    nb_b = x.shape[0]

    perm = _rope_perm()
    wqT = np.ascontiguousarray(Wq[perm].T).astype(bf16)  # [K, dout_perm]
    wkT = np.ascontiguousarray(Wk[perm].T).astype(bf16)
    wvT = np.ascontiguousarray(Wv.T).astype(bf16)
    woT = np.ascontiguousarray(Wo.T).astype(bf16)        # [d, c]

    # xt[p, kt, t] = x[b, t, kt*128+p]
    xts = []
    for b in range(nb_b):
        xT = x[b].T.reshape(KT, 128, t_len)
        xts.append(np.ascontiguousarray(xT.transpose(1, 0, 2)).astype(bf16))

    wq_s, wk_s, wv_s, wo_s = [], [], [], []
    for hg in range(4):
        sl = slice(hg * DS, (hg + 1) * DS)
        wq_s.append(np.ascontiguousarray(
            wqT[:, sl].reshape(KT, 128, DS).transpose(1, 0, 2)).astype(bf16))
        wk_s.append(np.ascontiguousarray(
            wkT[:, sl].reshape(KT, 128, DS).transpose(1, 0, 2)).astype(bf16))
        wv_s.append(np.ascontiguousarray(
            wvT[:, sl].reshape(KT, 128, DS).transpose(1, 0, 2)).astype(bf16))
        wo_s.append(np.ascontiguousarray(
            woT[sl].reshape(HS, 128, DM).transpose(1, 0, 2)).astype(bf16))

    inv = 1.0 / THETA ** (np.arange(0, HEAD_DIM, 2, dtype=np.float32) / HEAD_DIM)
    fr = np.outer(inv, np.arange(t_len, dtype=np.float32))  # [64, T]
    cosT = np.cos(fr).astype(np.float32)
    sinT = np.sin(fr).astype(np.float32)
    ccat = np.concatenate([cosT, cosT], axis=0).astype(bf16)   # [128, T]
    scat = np.concatenate([-sinT, sinT], axis=0).astype(bf16)  # [128, T]

    r = np.arange(128)[:, None]
    qc = np.arange(CH)[None, :]
    mask = np.stack([(r <= qc), (128 + r <= qc)], axis=1).astype(bf16)  # [128,2,256]

    in_maps = []
    for core in range(8):
        b, hg = core // 4, core % 4
        in_maps.append({
            "xt": xts[b], "wq": wq_s[hg], "wk": wk_s[hg], "wv": wv_s[hg],
            "wo": wo_s[hg], "ccat": ccat, "scat": scat, "mask": mask,
        })
    return in_maps


def kernel(x, Wq, Wk, Wv, Wo):
    global _NC, LAST_EXEC_NS
    from concourse.bass_utils import run_bass_kernel_spmd

    profile = bool(os.environ.get("KERNEL_PROFILE"))
    if profile:
        try:
            import hook_util
            hook_util.install()
            hook_util.patch_upload()
        except ImportError:
            profile = False

    in_maps = _prep_inputs(x, Wq, Wk, Wv, Wo)
    if _NC is None:
        _NC = _build_nc()

    kwargs = {}
    if profile:
        kwargs["tmpdir"] = os.environ.get("KERNEL_TRACE_DIR") or None
    res = run_bass_kernel_spmd(
        _NC, in_maps, core_ids=list(range(8)), trace=profile, **kwargs
    )
    LAST_EXEC_NS = res.exec_time_ns

    out = np.zeros((B, T, DM), dtype=np.float32)
    for core in range(8):
        out[core // 4] += res.results[core]["y"]
    return out


# revision 4
# speedup vs baseline: 1.1301x; 1.1301x over previous
"""Chunked sliding-window attention (B=2, T=8192, H=16, Dh=128, W=256) on 8
Trainium2 NeuronCores.

Sharding: 8 cores = 2 (batch) x 4 (head groups of 4 heads). Each core computes
q/k/v projections for its 512-wide slice of the 2048 projection dims, RoPE,
chunked attention for its 4 heads, and a partial output projection over its
512 rows of Wo^T. The host sums the 4 partial outputs per batch element.

Device layouts (host-prepared):
  xt   [128, 16, T]   x^T tiles: xt[p, kt, t] = x[b, t, kt*128+p]        (bf16)
  wq/wk[128, 16, 512] (Wq_perm)^T slice, rope-split row permutation      (bf16)
  wv   [128, 16, 512] Wv^T slice (unpermuted)                            (bf16)
  wo   [128, 4, 2048] Wo^T rows for this core's 512 dims                 (bf16)
  ccat [128, T]       [cos; cos] rope table (freq idx on partitions)     (bf16)
  scat [128, T]       [-sin; sin]                                        (bf16)
  mask [128, 2, 256]  transposed causal 0/1 masks for own-chunk kv tiles (bf16)

The rope row-permutation maps interleaved (re,im) pairs to split layout
(re block rows 0..63, im rows 64..127 per head); applied identically to q and
k it leaves scores invariant, and makes rope unit-stride on chip.

Attention is computed in transposed-score layout [kv, q]: softmax denominator
comes from an all-ones stationary matmul (broadcasts the per-q denominator
across all 128 partitions), masking is a 0/1 multiply after exp.
"""

import os

import numpy as np
import ml_dtypes

N_HEAD = 16
HEAD_DIM = 128
WINDOW = 256
THETA = 10000.0
B = 2
T = 8192
DM = 2048
KT = DM // 128      # 16 contraction tiles
HS = 4              # heads per core
DS = HS * HEAD_DIM  # 512 projection dims per core
BLK = 512           # tokens per pipeline block (2 chunks)
CH = WINDOW         # 256
SCALE = float(HEAD_DIM) ** -0.5

LAST_EXEC_NS = None
_NC = None

bf16 = ml_dtypes.bfloat16


def _build_nc(t_len=T):
    from contextlib import ExitStack

    import concourse.tile as tile
    from concourse import bacc, mybir

    fp32 = mybir.dt.float32
    b16 = mybir.dt.bfloat16

    nb = t_len // BLK
    nc = bacc.Bacc("TRN2", target_bir_lowering=False, debug=False)

    xt = nc.dram_tensor("xt", [128, KT, t_len], b16, kind="ExternalInput").ap()
    wq = nc.dram_tensor("wq", [128, KT, DS], b16, kind="ExternalInput").ap()
    wk = nc.dram_tensor("wk", [128, KT, DS], b16, kind="ExternalInput").ap()
    wv = nc.dram_tensor("wv", [128, KT, DS], b16, kind="ExternalInput").ap()
    wo = nc.dram_tensor("wo", [128, HS, DM], b16, kind="ExternalInput").ap()
    ccat = nc.dram_tensor("ccat", [128, t_len], b16, kind="ExternalInput").ap()
    scat = nc.dram_tensor("scat", [128, t_len], b16, kind="ExternalInput").ap()
    mask = nc.dram_tensor("mask", [128, 2, CH], b16, kind="ExternalInput").ap()
    y = nc.dram_tensor("y", [t_len, DM], fp32, kind="ExternalOutput").ap()

    Exp = mybir.ActivationFunctionType.Exp

    with tile.TileContext(nc) as tc, ExitStack() as ctx:
        const = ctx.enter_context(tc.tile_pool(name="const", bufs=1))
        xt_p = ctx.enter_context(tc.tile_pool(name="xtp", bufs=2))
        raw_p = ctx.enter_context(tc.tile_pool(name="rawp", bufs=3))
        swp_p = ctx.enter_context(tc.tile_pool(name="swpp", bufs=3))
        tmp_p = ctx.enter_context(tc.tile_pool(name="tmpp", bufs=3))
        qr_p = ctx.enter_context(tc.tile_pool(name="qrp", bufs=8))
        kr_p = ctx.enter_context(tc.tile_pool(name="krp", bufs=10))
        v_p = ctx.enter_context(tc.tile_pool(name="vp", bufs=10))
        e_p = ctx.enter_context(tc.tile_pool(name="ep", bufs=8))
        rc_p = ctx.enter_context(tc.tile_pool(name="rcp", bufs=4))
        ot_p = ctx.enter_context(tc.tile_pool(name="otp", bufs=16))
        y_p = ctx.enter_context(tc.tile_pool(name="yp", bufs=2))
        ps_big = ctx.enter_context(tc.tile_pool(name="psbig", bufs=2, space="PSUM"))
        ps_st = ctx.enter_context(tc.tile_pool(name="psst", bufs=3, space="PSUM"))
        ps_do = ctx.enter_context(tc.tile_pool(name="psdo", bufs=3, space="PSUM"))

        wq_sb = const.tile([128, KT, DS], b16)
        nc.sync.dma_start(wq_sb, wq)
        wk_sb = const.tile([128, KT, DS], b16)
        nc.sync.dma_start(wk_sb, wk)
        wv_sb = const.tile([128, KT, DS], b16)
        nc.sync.dma_start(wv_sb, wv)
        wo_sb = const.tile([128, HS, DM], b16)
        nc.sync.dma_start(wo_sb, wo)
        ccat_sb = const.tile([128, t_len], b16)
        nc.sync.dma_start(ccat_sb, ccat)
        scat_sb = const.tile([128, t_len], b16)
        nc.sync.dma_start(scat_sb, scat)
        mask_sb = const.tile([128, 2, CH], b16)
        nc.sync.dma_start(mask_sb, mask)
        ones_sb = const.tile([128, 128], b16)
        nc.vector.memset(ones_sb, 1.0)

        prev_k = [None] * HS
        prev_v = [None, None]
        for blk in range(nb):
            t0 = blk * BLK
            xt_sb = xt_p.tile([128, KT, BLK], b16, tag="xt")
            nc.sync.dma_start(xt_sb, xt[:, :, t0 : t0 + BLK])

            c_sl = ccat_sb[:, t0 : t0 + BLK]
            s_sl = scat_sb[:, t0 : t0 + BLK]
            cur_q = []
            cur_k = []
            for h in range(HS):
                for w_sb, dst in ((wq_sb, cur_q), (wk_sb, cur_k)):
                    ps = ps_big.tile([128, BLK], fp32, tag="psbig")
                    for k in range(KT):
                        nc.tensor.matmul(
                            ps,
                            lhsT=w_sb[:, k, h * 128 : (h + 1) * 128],
                            rhs=xt_sb[:, k, :],
                            start=(k == 0),
                            stop=(k == KT - 1),
                        )
                    raw = raw_p.tile([128, BLK], b16, tag="raw")
                    nc.scalar.copy(raw, ps)
                    # swap the (re, im) halves via SBUF->SBUF DMA (DVE lanes
                    # cannot cross partitions)
                    swp = swp_p.tile([128, BLK], b16, tag="swp")
                    nc.sync.dma_start(swp[0:64, :], raw[64:128, :])
                    nc.sync.dma_start(swp[64:128, :], raw[0:64, :])
                    t1 = tmp_p.tile([128, BLK], b16, tag="t1")
                    nc.vector.tensor_mul(t1, raw, c_sl)
                    t2 = tmp_p.tile([128, BLK], b16, tag="t2")
                    nc.vector.tensor_mul(t2, swp, s_sl)
                    if dst is cur_q:
                        rot = qr_p.tile([128, BLK], b16, tag="qr")
                    else:
                        rot = kr_p.tile([128, BLK], b16, tag="kr")
                    nc.vector.tensor_add(rot, t1, t2)
                    dst.append(rot)

            cur_v = []
            for tt in range(4):
                ps = ps_big.tile([128, BLK], fp32, tag="psbig")
                for k in range(KT):
                    nc.tensor.matmul(
                        ps,
                        lhsT=xt_sb[:, k, tt * 128 : (tt + 1) * 128],
                        rhs=wv_sb[:, k, :],
                        start=(k == 0),
                        stop=(k == KT - 1),
                    )
                vt = v_p.tile([128, DS], b16, tag="v")
                nc.vector.tensor_copy(out=vt, in_=ps)
                cur_v.append(vt)

            ot_tiles = {}
            for ci in range(2):
                c = 2 * blk + ci
                qoff = ci * CH
                js = [2, 3] if c == 0 else [0, 1, 2, 3]
                for h in range(HS):
                    q_sl = cur_q[h][:, qoff : qoff + CH]
                    es = []
                    for j in js:
                        if j < 2:
                            if ci == 1:
                                ksrc = cur_k[h][:, j * 128 : (j + 1) * 128]
                            else:
                                ksrc = prev_k[h][:, CH + j * 128 : CH + (j + 1) * 128]
                        else:
                            ksrc = cur_k[h][:, qoff + (j - 2) * 128 : qoff + (j - 1) * 128]
                        st = ps_st.tile([128, CH], fp32, tag="st")
                        nc.tensor.matmul(st, lhsT=ksrc, rhs=q_sl, start=True, stop=True)
                        e = e_p.tile([128, CH], b16, tag="e")
                        nc.scalar.activation(e, st, Exp, scale=SCALE)
                        if j >= 2:
                            nc.vector.tensor_mul(e, e, mask_sb[:, j - 2, :])
                        es.append((j, e))
                    dn = ps_do.tile([128, CH], fp32, tag="do")
                    for i, (j, e) in enumerate(es):
                        nc.tensor.matmul(
                            dn, lhsT=ones_sb, rhs=e,
                            start=(i == 0), stop=(i == len(es) - 1),
                        )
                    ou = ps_do.tile([128, CH], fp32, tag="do")
                    for i, (j, e) in enumerate(es):
                        if j < 2:
                            vsrc = cur_v[j] if ci == 1 else prev_v[j]
                        else:
                            vsrc = cur_v[2 * ci + (j - 2)]
                        nc.tensor.matmul(
                            ou, lhsT=vsrc[:, h * 128 : (h + 1) * 128], rhs=e,
                            start=(i == 0), stop=(i == len(es) - 1),
                        )
                    rc = rc_p.tile([128, CH], fp32, tag="rc")
                    nc.vector.reciprocal_approx_fast(out=rc, in_=dn)
                    ot = ot_p.tile([128, CH], b16, tag="ot")
                    nc.vector.tensor_mul(ot, ou, rc)
                    ot_tiles[(h, ci)] = ot

            for tt in range(4):
                ci, sub = tt // 2, tt % 2
                ysb = y_p.tile([128, DM], fp32, tag="y")
                for ct in range(4):
                    yps = ps_big.tile([128, 512], fp32, tag="psbig")
                    for h in range(HS):
                        nc.tensor.matmul(
                            yps,
                            lhsT=ot_tiles[(h, ci)][:, sub * 128 : (sub + 1) * 128],
                            rhs=wo_sb[:, h, ct * 512 : (ct + 1) * 512],
                            start=(h == 0),
                            stop=(h == HS - 1),
                        )
                    if ct % 2 == 0:
                        nc.scalar.copy(ysb[:, ct * 512 : (ct + 1) * 512], yps)
                    else:
                        nc.vector.tensor_copy(
                            out=ysb[:, ct * 512 : (ct + 1) * 512], in_=yps
                        )
                nc.sync.dma_start(y[t0 + tt * 128 : t0 + (tt + 1) * 128, :], ysb)

            prev_k = cur_k
            prev_v = cur_v[2:4]

    nc.compile()
    return nc


def _rope_perm():
    perm = np.empty(DM, np.int64)
    for h in range(N_HEAD):
        base = h * HEAD_DIM
        perm[base : base + 64] = base + 2 * np.arange(64)
        perm[base + 64 : base + 128] = base + 2 * np.arange(64) + 1
    return perm


def _prep_inputs(x, Wq, Wk, Wv, Wo, t_len=T):
    """Build per-core in_maps. Cores 0-3: batch 0, head groups 0-3; 4-7: batch 1."""
    x = np.asarray(x, dtype=np.float32)
    Wq = np.asarray(Wq, dtype=np.float32)
    Wk = np.asarray(Wk, dtype=np.float32)
    Wv = np.asarray(Wv, dtype=np.float32)
    Wo = np.asarray(Wo, dtype=np.float32)
    nb_b = x.shape[0]

    perm = _rope_perm()
    wqT = np.ascontiguousarray(Wq[perm].T).astype(bf16)  # [K, dout_perm]
    wkT = np.ascontiguousarray(Wk[perm].T).astype(bf16)
    wvT = np.ascontiguousarray(Wv.T).astype(bf16)
    woT = np.ascontiguousarray(Wo.T).astype(bf16)        # [d, c]

    # xt[p, kt, t] = x[b, t, kt*128+p]
    xts = []
    for b in range(nb_b):
        xT = x[b].T.reshape(KT, 128, t_len)
        xts.append(np.ascontiguousarray(xT.transpose(1, 0, 2)).astype(bf16))

    wq_s, wk_s, wv_s, wo_s = [], [], [], []
    for hg in range(4):
        sl = slice(hg * DS, (hg + 1) * DS)
        wq_s.append(np.ascontiguousarray(
            wqT[:, sl].reshape(KT, 128, DS).transpose(1, 0, 2)).astype(bf16))
        wk_s.append(np.ascontiguousarray(
            wkT[:, sl].reshape(KT, 128, DS).transpose(1, 0, 2)).astype(bf16))
        wv_s.append(np.ascontiguousarray(
            wvT[:, sl].reshape(KT, 128, DS).transpose(1, 0, 2)).astype(bf16))
        wo_s.append(np.ascontiguousarray(
            woT[sl].reshape(HS, 128, DM).transpose(1, 0, 2)).astype(bf16))

    inv = 1.0 / THETA ** (np.arange(0, HEAD_DIM, 2, dtype=np.float32) / HEAD_DIM)
    fr = np.outer(inv, np.arange(t_len, dtype=np.float32))  # [64, T]
    cosT = np.cos(fr).astype(np.float32)
    sinT = np.sin(fr).astype(np.float32)
    ccat = np.concatenate([cosT, cosT], axis=0).astype(bf16)   # [128, T]
    scat = np.concatenate([-sinT, sinT], axis=0).astype(bf16)  # [128, T]

    r = np.arange(128)[:, None]
    qc = np.arange(CH)[None, :]
    mask = np.stack([(r <= qc), (128 + r <= qc)], axis=1).astype(bf16)  # [128,2,256]

    in_maps = []
    for core in range(8):
        b, hg = core // 4, core % 4
        in_maps.append({
            "xt": xts[b], "wq": wq_s[hg], "wk": wk_s[hg], "wv": wv_s[hg],
            "wo": wo_s[hg], "ccat": ccat, "scat": scat, "mask": mask,
        })
    return in_maps


def kernel(x, Wq, Wk, Wv, Wo):
    global _NC, LAST_EXEC_NS
    from concourse.bass_utils import run_bass_kernel_spmd

    profile = bool(os.environ.get("KERNEL_PROFILE"))
    if profile:
        try:
            import hook_util
            hook_util.install()
            hook_util.patch_upload()
        except ImportError:
            profile = False

    in_maps = _prep_inputs(x, Wq, Wk, Wv, Wo)
    if _NC is None:
        _NC = _build_nc()

    kwargs = {}
    if profile:
        kwargs["tmpdir"] = os.environ.get("KERNEL_TRACE_DIR") or None
    res = run_bass_kernel_spmd(
        _NC, in_maps, core_ids=list(range(8)), trace=profile, **kwargs
    )
    LAST_EXEC_NS = res.exec_time_ns

    out = np.zeros((B, T, DM), dtype=np.float32)
    for core in range(8):
        out[core // 4] += res.results[core]["y"]
    return out
